# revision 1
# baseline (speedup 1.0000x reference)
"""Trainium2 Bass kernel for nn_DenseContrastLoss.

Strategy (data-parallel over instances, 8 cores):
  - Host: transpose feats to [N, 784, 256] (pixel-major rows), shard 13
    instances per core, build flat gather indices for the 96 sampled
    pixels per instance (32 anchor + 32 pos + 32 neg).
  - Device (per core): dma_gather the 1248 needed pixel-vectors straight
    from HBM (the rest of feats is never touched), PE-transpose to
    channel-major, run the 2-layer 1x1-conv projection head as matmuls,
    L2-normalize via square/colsum/exp(-0.5 ln), form the 32x32
    anchor-pos and anchor-neg similarity matrices per instance with PE,
    and finish the InfoNCE-style loss with DVE/ACT ops. Outputs 13
    per-instance losses.
  - Host: validity mask from gt_mask areas, masked mean, * LOSS_WEIGHT.
"""

import os
import sys

import numpy as np

if "/opt/trn_rl_repo" not in sys.path:
    sys.path.insert(0, "/opt/trn_rl_repo")

import concourse.bass as bass
import concourse.tile as tile
from concourse import bacc, library_config, mybir
from concourse.bass_utils import run_bass_kernel_spmd

F32 = mybir.dt.float32
F32R = mybir.dt.float32r
I16 = mybir.dt.int16

TAU = 0.07
LOSS_WEIGHT = 1.2
NUM_SAMPLES = 32
C = 256
SIDE = 28
PIX = SIDE * SIDE  # 784
N_INST = 100
N_CORES = 8
NI = 13                      # instances per core (8*13 = 104 >= 100)
SAMP = 3 * NUM_SAMPLES       # 96 sampled pixels per instance
STOT = NI * SAMP             # 1248
NPAD = 1280                  # gather count, multiple of 128
NSLOT = NPAD // 128          # 10
IDXW = NPAD // 16            # 80
CHUNK = 416                  # 1248 = 3 * 416, fits one PSUM bank (fp32)
NCH = STOT // CHUNK          # 3

# float32r shares fp32 storage but streams 4x faster through the PE at
# N>=256 (relaxed-precision multiply mode). Tiles consumed by fp32r
# matmuls must be written as f32r by their producers (BIR verifier rule).
MMDT = F32R if os.environ.get("PROJ_DT", "f32r") == "f32r" else F32


def _build_nc():
    nc = bacc.Bacc("TRN2", target_bir_lowering=False, num_swdge_queues=4)
    featsT = nc.declare_dram_parameter("featsT", [NI * PIX, C], F32, isOutput=False)
    idxw = nc.declare_dram_parameter("idxw", [128, IDXW], I16, isOutput=False)
    w1t = nc.declare_dram_parameter("w1t", [C, C], MMDT, isOutput=False)
    w2t = nc.declare_dram_parameter("w2t", [C, C], MMDT, isOutput=False)
    b1 = nc.declare_dram_parameter("b1", [C], F32, isOutput=False)
    b2 = nc.declare_dram_parameter("b2", [C], F32, isOutput=False)
    identw = nc.declare_dram_parameter("identw", [128, 128], F32, isOutput=False)
    blockr = nc.declare_dram_parameter("blockr", [128, 2], F32, isOutput=False)
    loss = nc.declare_dram_parameter("loss", [14], F32, isOutput=True)

    AT = mybir.ActivationFunctionType
    ALU = mybir.AluOpType
    PSUM = bass.MemorySpace.PSUM

    with tile.TileContext(nc) as tc:
        with tc.tile_pool(name="singles", bufs=1) as singles:
            # Get the GPSIMD library load issued as early as possible: the
            # ~10us Q7 IRAM load gates the gathers, which gate everything.
            nc.gpsimd.load_library(library_config.mlp)
            # Preload the one ACT table set that covers every function this
            # kernel uses (exp, ln, copy, square, relu, identity) so the
            # auto-inserted per-transition loads (1.3us each) never fire.
            nc.scalar.add_instruction(
                mybir.InstLoadActFuncSet(
                    name=nc.get_next_instruction_name(),
                    ins=[],
                    outs=[],
                    act_func_set_id=6,  # natural_log_exp_and_others
                )
            )

            idx_s = singles.tile([128, IDXW], I16)
            nc.sync.dma_start(out=idx_s[:], in_=idxw[:, :])

            W1 = singles.tile([128, 2, C], MMDT)
            nc.sync.dma_start(out=W1[:], in_=w1t.rearrange("(k p) d -> p k d", p=128))
            W2 = singles.tile([128, 2, C], MMDT)
            nc.sync.dma_start(out=W2[:], in_=w2t.rearrange("(k p) d -> p k d", p=128))
            B1 = singles.tile([128, 2], F32)
            nc.sync.dma_start(out=B1[:], in_=b1.rearrange("(m p) -> p m", p=128))
            B2 = singles.tile([128, 2], F32)
            nc.sync.dma_start(out=B2[:], in_=b2.rearrange("(m p) -> p m", p=128))
            ident = singles.tile([128, 128], F32)
            nc.sync.dma_start(out=ident[:], in_=identw[:, :])
            blockt = singles.tile([128, 2], F32)
            nc.sync.dma_start(out=blockt[:], in_=blockr[:, :])

            ones32 = singles.tile([32, 1], F32)
            nc.vector.memset(ones32[:], 1.0)
            onescf = singles.tile([128, 1], F32)
            nc.vector.memset(onescf[:], 1.0)
            onesrf = singles.tile([1, 128], F32)
            nc.vector.memset(onesrf[:], 1.0)
            # memset can't write f32r; round 1.0 through an ACT copy instead
            onesc = singles.tile([128, 1], MMDT)
            nc.scalar.copy(out=onesc[:], in_=onescf[:])
            onesr = singles.tile([1, 128], MMDT)
            nc.scalar.copy(out=onesr[:], in_=onesrf[:])

            with tc.tile_pool(name="big", bufs=1) as big:
                # ---- gather the sampled pixel-vectors from HBM ----
                # The SWDGE descriptor ring holds at most 1024 descriptors
                # per instruction; split across the two SWDGE queues so the
                # two Q7 core-pairs generate descriptors in parallel.
                g = big.tile([128, NSLOT, C], F32)
                base = 0
                for q, cnt in enumerate((384, 384, 384, 128)):
                    s0, s1 = base // 128, (base + cnt) // 128
                    nc.gpsimd.dma_gather(
                        g[:, s0:s1, :], featsT[:, :],
                        idx_s[:, base // 16 : (base + cnt) // 16],
                        cnt, cnt, C, queue_num=q,
                    )
                    base += cnt

                # ---- PE warm-up during the gpsimd library-load window ----
                nwarm = int(os.environ.get("NWARM", "64"))
                if nwarm:
                    with tc.tile_pool(name="warmp", bufs=1, space=PSUM) as warmp:
                        wt = warmp.tile([128, 128], F32, tag="warm")
                        for _ in range(nwarm):
                            nc.tensor.transpose(wt[:], ident[:], ident[:])

                # ---- transpose to channel-major Gt[c, s] ----
                gt = [big.tile([128, NPAD], MMDT, tag=f"gt{h}", name=f"gt{h}")
                      for h in range(2)]
                # transpose groups aligned to gather-queue slot ranges so
                # each group waits on exactly one queue's DMA completion
                with tc.tile_pool(name="tpp", bufs=3, space=PSUM) as tpp:
                    for s0, nsl in ((0, 3), (3, 3), (6, 3), (9, 1)):
                        for h in range(2):
                            tp = tpp.tile([128, 384], F32, tag="tp")
                            for j in range(nsl):
                                nc.tensor.transpose(
                                    tp[:, 128 * j : 128 * (j + 1)],
                                    g[:, s0 + j, 128 * h : 128 * (h + 1)],
                                    ident[:],
                                )
                            nc.vector.tensor_copy(
                                out=gt[h][:, 128 * s0 : 128 * (s0 + nsl)],
                                in_=tp[:, : 128 * nsl],
                            )

                # ---- projection head: P = w2 @ relu(w1 @ G + b1) + b2 ----
                hs = [big.tile([128, STOT], MMDT, tag=f"hs{m}", name=f"hs{m}")
                      for m in range(2)]
                ps = [big.tile([128, STOT], F32, tag=f"ps{m}", name=f"ps{m}")
                      for m in range(2)]
                qs = [big.tile([128, STOT], MMDT, tag=f"qs{m}", name=f"qs{m}")
                      for m in range(2)]
                pn = [big.tile([128, STOT], MMDT, tag=f"pn{m}", name=f"pn{m}")
                      for m in range(2)]

                with tc.tile_pool(name="mmp", bufs=3, space=PSUM) as mmp:
                    for m in range(2):
                        for ch in range(NCH):
                            sl = slice(CHUNK * ch, CHUNK * (ch + 1))
                            hp = mmp.tile([128, CHUNK], F32, tag="hp")
                            for k in range(2):
                                nc.tensor.matmul(
                                    hp[:],
                                    W1[:, k, 128 * m : 128 * (m + 1)],
                                    gt[k][:, sl],
                                    start=(k == 0),
                                    stop=(k == 1),
                                )
                            # relu(x + b1), alternating DVE / ACT
                            if ch % 2 == 0:
                                nc.vector.tensor_scalar(
                                    out=hs[m][:, sl],
                                    in0=hp[:],
                                    scalar1=B1[:, m : m + 1],
                                    scalar2=0.0,
                                    op0=ALU.add,
                                    op1=ALU.max,
                                )
                            else:
                                nc.scalar.activation(
                                    out=hs[m][:, sl], in_=hp[:], func=AT.Relu,
                                    bias=B1[:, m : m + 1],
                                )
                    for m in range(2):
                        for ch in range(NCH):
                            sl = slice(CHUNK * ch, CHUNK * (ch + 1))
                            pp = mmp.tile([128, CHUNK], F32, tag="hp")
                            for k in range(2):
                                nc.tensor.matmul(
                                    pp[:],
                                    W2[:, k, 128 * m : 128 * (m + 1)],
                                    hs[k][:, sl],
                                    start=(k == 0),
                                    stop=(k == 1),
                                )
                            if ch % 2 == 1:
                                nc.vector.tensor_scalar_add(
                                    out=ps[m][:, sl], in0=pp[:],
                                    scalar1=B2[:, m : m + 1],
                                )
                            else:
                                nc.scalar.activation(
                                    out=ps[m][:, sl], in_=pp[:], func=AT.Identity,
                                    bias=B2[:, m : m + 1],
                                )
                            if ch % 2 == 0:
                                nc.vector.tensor_mul(
                                    out=qs[m][:, sl], in0=ps[m][:, sl],
                                    in1=ps[m][:, sl],
                                )
                            else:
                                nc.scalar.square(out=qs[m][:, sl], in_=ps[m][:, sl])

                # ---- per-sample rn = (tau * ||p||^2)^-0.5 row ----
                rrow = big.tile([1, STOT], MMDT)
                with (
                    tc.tile_pool(name="nsqp", bufs=2, space=PSUM) as nsqp,
                    tc.tile_pool(name="rnp", bufs=2, space=PSUM) as rnp,
                    tc.tile_pool(name="simp", bufs=1, space=PSUM) as simp,
                ):
                    for ch in range(NCH):
                        sl = slice(CHUNK * ch, CHUNK * (ch + 1))
                        nsq = nsqp.tile([1, CHUNK], F32, tag="nsq")
                        for m in range(2):
                            nc.tensor.matmul(
                                nsq[:],
                                onesc[:],
                                qs[m][:, sl],
                                start=(m == 0),
                                stop=(m == 1),
                            )
                        # (tau * nsq)^-0.5 = exp(-0.5 * ln(tau * nsq)); Ln and
                        # Exp share one ACT table set (natural_log_exp_and_others)
                        lnt = big.tile([1, CHUNK], F32, tag="lnt", name="lnt", bufs=2)
                        nc.scalar.activation(
                            out=lnt[:], in_=nsq[:], func=AT.Ln, scale=float(TAU)
                        )
                        nc.scalar.activation(
                            out=rrow[:, sl], in_=lnt[:], func=AT.Exp, scale=-0.5
                        )

                    # ---- normalize columns of P ----
                    for ch in range(NCH):
                        sl = slice(CHUNK * ch, CHUNK * (ch + 1))
                        rrep = rnp.tile([128, CHUNK], F32, tag="rrep")
                        nc.tensor.matmul(
                            rrep[:], onesr[:], rrow[:, sl], start=True, stop=True
                        )
                        for m in range(2):
                            nc.vector.tensor_mul(
                                out=pn[m][:, sl], in0=ps[m][:, sl], in1=rrep[:]
                            )

                    # ---- similarity matrices ----
                    # One N=64 matmul per (instance, k-tile) computes
                    # [sim_ap | sim_an] together (pos and neg columns are
                    # adjacent in pn). Two PSUM banks: instances 0-6 / 7-12.
                    GA, GB = 7, NI - 7
                    sa = simp.tile([32, GA * 64], F32, tag="sa")
                    sb = simp.tile([32, GB * 64], F32, tag="sb")
                    for n in range(NI):
                        a0 = SAMP * n
                        dst = sa if n < GA else sb
                        gi = n if n < GA else n - GA
                        osl = slice(64 * gi, 64 * (gi + 1))
                        for k in range(2):
                            nc.tensor.matmul(
                                dst[:, osl],
                                pn[k][:, a0 : a0 + 32],
                                pn[k][:, a0 + 32 : a0 + 96],
                                start=(k == 0),
                                stop=(k == 1),
                            )

                    # ---- softmax-style loss on [32, G, 64] views ----
                    lp = simp.tile([GA, 2], F32, tag="lp")
                    nc.vector.memset(lp[:], 0.0)
                    for half, (st, G) in enumerate(((sa, GA), (sb, GB))):
                        def v4(ap, inner=64, off=0):
                            v = ap.rearrange("p (n m) -> p n m", n=G)
                            return v[:, :, off : off + inner]

                        ap3 = v4(st[:], 32, 0)
                        sm = big.tile([32, G], F32, tag="sm", name="sm", bufs=2)
                        nc.vector.reduce_max(
                            out=sm[:], in_=ap3, axis=mybir.AxisListType.X
                        )
                        smb2 = sm[:].unsqueeze(-1).broadcast_to([32, G, 64])
                        dd = big.tile([32, G * 64], F32, tag="dd", name="dd", bufs=2)
                        nc.vector.tensor_sub(
                            out=dd[:].rearrange("p (n m) -> p n m", n=G),
                            in0=st[:].rearrange("p (n m) -> p n m", n=G),
                            in1=smb2,
                        )
                        ee = big.tile([32, G * 64], F32, tag="ee", name="ee", bufs=2)
                        nc.scalar.activation(out=ee[:], in_=dd[:], func=AT.Exp)

                        ssum = big.tile([32, G], F32, tag="ssum", name="ssum", bufs=2)
                        nc.vector.reduce_sum(
                            out=ssum[:], in_=v4(ee[:], 32, 32),
                            axis=mybir.AxisListType.X,
                        )
                        ssb = ssum[:].unsqueeze(-1).broadcast_to([32, G, 32])

                        tt = big.tile([32, G * 32], F32, tag="tt", name="tt", bufs=2)
                        nc.vector.tensor_add(
                            out=tt[:].rearrange("p (n m) -> p n m", n=G),
                            in0=v4(ee[:], 32, 0), in1=ssb,
                        )
                        lg = big.tile([32, G * 32], F32, tag="lg", name="lg", bufs=2)
                        nc.scalar.activation(out=lg[:], in_=tt[:], func=AT.Ln)
                        ctb = big.tile([32, G * 32], F32, tag="ctb", name="ctb", bufs=2)
                        nc.vector.tensor_sub(
                            out=ctb[:].rearrange("p (n m) -> p n m", n=G),
                            in0=lg[:].rearrange("p (n m) -> p n m", n=G),
                            in1=v4(dd[:], 32, 0),
                        )
                        rowr = big.tile([32, G], F32, tag="rowr", name="rowr", bufs=2)
                        nc.vector.reduce_sum(
                            out=rowr[:],
                            in_=ctb[:].rearrange("p (n m) -> p n m", n=G),
                            axis=mybir.AxisListType.X,
                        )
                        nc.tensor.matmul(
                            lp[:G, half : half + 1], rowr[:], ones32[:],
                            start=True, stop=True,
                        )

                    lout = big.tile([GA, 2], F32)
                    nc.scalar.mul(
                        out=lout[:], in_=lp[:], mul=1.0 / (NUM_SAMPLES * NUM_SAMPLES)
                    )
                    nc.sync.dma_start(
                        out=loss.rearrange("(a b) -> a b", b=2), in_=lout[:]
                    )

    nc.compile()
    return nc


_NC_CACHE = None


def _get_nc():
    global _NC_CACHE
    if _NC_CACHE is None:
        _NC_CACHE = _build_nc()
    return _NC_CACHE


def _host_prep(feats, w1, b1, w2, b2, anchor_inds, pos_inds, neg_inds):
    """Build the 8 per-core input maps."""
    n = feats.shape[0]
    ntot = N_CORES * NI
    # pixel-major feats, padded with copies of instance 0
    ft = np.asarray(feats, dtype=np.float32).reshape(n, C, PIX)
    ft = np.transpose(ft, (0, 2, 1))  # [N, 784, C]
    if ntot > n:
        pad = np.broadcast_to(ft[0], (ntot - n,) + ft.shape[1:])
        ft = np.concatenate([ft, pad], axis=0)
    ft = np.ascontiguousarray(ft.reshape(N_CORES, NI * PIX, C))

    def flat(inds):
        inds = np.asarray(inds)
        f = inds[..., 0].astype(np.int64) * SIDE + inds[..., 1].astype(np.int64)
        if ntot > n:
            f = np.concatenate(
                [f, np.broadcast_to(f[0], (ntot - n,) + f.shape[1:])], axis=0
            )
        return f  # [ntot, 32]

    af, pf, nf = flat(anchor_inds), flat(pos_inds), flat(neg_inds)
    samp = np.concatenate([af, pf, nf], axis=1)  # [ntot, 96]
    base = (np.arange(ntot) % NI)[:, None] * PIX
    rows = (samp + base).reshape(N_CORES, STOT)  # [8, 1248]

    w1t = np.ascontiguousarray(np.asarray(w1, dtype=np.float32).T)
    w2t = np.ascontiguousarray(np.asarray(w2, dtype=np.float32).T)
    b1 = np.ascontiguousarray(np.asarray(b1, dtype=np.float32))
    b2 = np.ascontiguousarray(np.asarray(b2, dtype=np.float32))
    identw = np.eye(128, dtype=np.float32)
    blockr = np.zeros((128, 2), dtype=np.float32)
    for b in range(2):
        blockr[64 * b : 64 * b + 32, b] = 1.0

    in_maps = []
    for c in range(N_CORES):
        idx = np.zeros(NPAD, dtype=np.int16)
        idx[:STOT] = rows[c]
        wrapped = np.ascontiguousarray(idx.reshape(IDXW, 16).T)  # [16, 80]
        idx128 = np.ascontiguousarray(np.tile(wrapped, (8, 1)))  # [128, 80]
        in_maps.append(
            {
                "featsT": ft[c],
                "idxw": idx128,
                "w1t": w1t,
                "w2t": w2t,
                "b1": b1,
                "b2": b2,
                "identw": identw,
                "blockr": blockr,
            }
        )
    return in_maps


def _finalize(loss_per, gt_mask):
    gt = np.asarray(gt_mask)
    area = gt.reshape(gt.shape[0], -1).sum(axis=1)
    valid = (area > NUM_SAMPLES) & (area < PIX - NUM_SAMPLES)
    n_valid = np.float32(valid.sum())
    if n_valid > 0:
        total = np.float32(np.where(valid, loss_per, 0.0).astype(np.float32).sum())
        out = total / max(n_valid, np.float32(1.0))
    else:
        out = np.float32(0.0)
    return np.float32(out * np.float32(LOSS_WEIGHT))


def kernel(feats, w1, b1, w2, b2, gt_mask, anchor_inds, pos_inds, neg_inds,
           _results_hook=None):
    nc = _get_nc()
    in_maps = _host_prep(feats, w1, b1, w2, b2, anchor_inds, pos_inds, neg_inds)
    res = run_bass_kernel_spmd(nc, in_maps, list(range(N_CORES)))
    if _results_hook is not None:
        _results_hook(res)
    parts = []
    for c in range(N_CORES):
        lo = res.results[c]["loss"].reshape(7, 2)
        parts.append(np.concatenate([lo[:, 0], lo[: NI - 7, 1]]))
    loss_per = np.concatenate(parts)[:N_INST]
    return _finalize(loss_per, gt_mask)



# revision 4
# speedup vs baseline: 1.7381x; 1.7381x over previous
"""Trainium2 Bass kernel for nn_DenseContrastLoss (v2).

Strategy (data-parallel over instances, 8 cores, 13 instances each):
  - Host: the two 1x1-conv projections are per-pixel, so only the 96
    sampled pixels per instance ever matter.  Gather them on the host
    (trivial numpy fancy-indexing) and ship a dense channel-major
    [256 x 1248] bf16 matrix per core -- no GPSIMD gathers, no on-device
    transposes, ~0.65 MB of input instead of 10.4 MB.
  - Device: 2-layer projection head as bf16 matmuls (PSUM f32 accum),
    L2 normalization via square/ones-matmul-colsum/exp(-0.5 ln(tau x)),
    then per-instance 32x64 [sim_ap | sim_an] similarity blocks packed
    4 instances per 128 PSUM partitions (PE column tiling: a 32-row
    matmul writes at partition offset 32q).  The InfoNCE loss skips the
    max-subtraction (it cancels exactly; |logit| <= 1/tau so exp stays
    finite) and runs on fully dense [128, 256] tiles.  A final mask
    matmul reduces over the 32 anchors; host divides by 32*32, applies
    the validity mask and the loss weight.
"""

import os
import sys

import numpy as np

if "/opt/trn_rl_repo" not in sys.path:
    sys.path.insert(0, "/opt/trn_rl_repo")

import concourse.bass as bass
import concourse.tile as tile
from concourse import bacc, mybir
from concourse.bass_utils import run_bass_kernel_spmd

try:
    from ml_dtypes import bfloat16 as np_bf16
except ImportError:  # ml_dtypes ships with jax
    import jax.numpy as _jnp

    np_bf16 = _jnp.bfloat16

F32 = mybir.dt.float32
BF16 = mybir.dt.bfloat16

TAU = 0.07
LOSS_WEIGHT = 1.2
NUM_SAMPLES = 32
C = 256
SIDE = 28
PIX = SIDE * SIDE
N_INST = 100
N_CORES = 8
NI = 13                      # instances per core (8*13 = 104 >= 100)
SAMP = 3 * NUM_SAMPLES       # 96 sampled pixels per instance
S = NI * SAMP                # 1248 columns per core
CHUNK = 416                  # 1248 = 3*416, one PSUM bank in fp32
NCH = S // CHUNK             # 3
NWARM = int(os.environ.get("NWARM", "8"))


def _build_nc():
    nc = bacc.Bacc("TRN2", target_bir_lowering=False)
    gtd = nc.declare_dram_parameter("gtd", [128, NCH, 2, CHUNK], BF16, isOutput=False)
    w1d = nc.declare_dram_parameter("w1d", [128, 2, 2, 128], BF16, isOutput=False)
    w2d = nc.declare_dram_parameter("w2d", [128, 2, 2, 128], BF16, isOutput=False)
    biasd = nc.declare_dram_parameter("biasd", [128, 4], F32, isOutput=False)
    maskd = nc.declare_dram_parameter("maskd", [128, 16], F32, isOutput=False)
    lossd = nc.declare_dram_parameter("loss", [16, 4], F32, isOutput=True)

    AT = mybir.ActivationFunctionType
    ALU = mybir.AluOpType
    PSUM = bass.MemorySpace.PSUM
    X = mybir.AxisListType.X

    with tile.TileContext(nc) as tc:
        with tc.tile_pool(name="singles", bufs=1) as sg:
            W1 = sg.tile([128, 2, 2, 128], BF16)
            W2 = sg.tile([128, 2, 2, 128], BF16)
            BIA = sg.tile([128, 4], F32)
            MSK = sg.tile([128, 16], F32)
            gt = [sg.tile([128, 2, CHUNK], BF16, name=f"gt{c}") for c in range(NCH)]
            wones = sg.tile([128, 64], F32)
            ones128 = sg.tile([128, 128], BF16)

            nc.vector.memset(wones[:], 1.0)
            nc.vector.memset(ones128[:], 1.0)

            # Input DMAs split across the two HWDGE queues (sync + scalar),
            # ordered so the first chunks land first.
            nc.sync.dma_start(out=gt[0][:], in_=gtd[:, 0])
            nc.scalar.dma_start(out=W1[:], in_=w1d[:, :, :, :])
            nc.sync.dma_start(out=BIA[:], in_=biasd[:, :])
            nc.scalar.dma_start(out=gt[1][:], in_=gtd[:, 1])
            nc.sync.dma_start(out=gt[2][:], in_=gtd[:, 2])
            nc.scalar.dma_start(out=W2[:], in_=w2d[:, :, :, :])
            nc.sync.dma_start(out=MSK[:], in_=maskd[:, :])
            # One ACT table set covers every function used here
            # (relu, identity, square, ln, exp, copy).
            nc.scalar.add_instruction(
                mybir.InstLoadActFuncSet(
                    name=nc.get_next_instruction_name(),
                    ins=[],
                    outs=[],
                    act_func_set_id=6,  # natural_log_exp_and_others
                )
            )

            # PE warm-up during the DMA window (the tensor engine clock
            # ramps with sustained use).
            with tc.tile_pool(name="warmp", bufs=1, space=PSUM) as warmp:
                wt = warmp.tile([64, 64], F32, tag="warm")
                for _ in range(NWARM):
                    nc.tensor.matmul(
                        wt[:], wones[:, :64], wones[:, :64], start=True, stop=True
                    )

            with (
                tc.tile_pool(name="big", bufs=1) as big,
                tc.tile_pool(name="mmp", bufs=3, space=PSUM) as mmp,
                tc.tile_pool(name="nsqp", bufs=2, space=PSUM) as nsqp,
                tc.tile_pool(name="simp", bufs=1, space=PSUM) as simp,
                tc.tile_pool(name="lpp", bufs=1, space=PSUM) as lpp,
            ):
                HS = big.tile([128, 2, S], BF16, name="HS")
                PS = big.tile([128, 2, S], BF16, name="PS")
                QS = big.tile([128, 2, S], BF16, name="QS")
                PN = big.tile([128, 2, S], BF16, name="PN")
                RR = big.tile([128, S], BF16, name="RR")
                sp = simp.tile([128, 256], F32, tag="sp")

                def sl(c):
                    return slice(CHUNK * c, CHUNK * (c + 1))

                def l1(c, m, eng):
                    pp = mmp.tile([128, CHUNK], F32, tag="pp")
                    for k in (0, 1):
                        nc.tensor.matmul(
                            pp[:], W1[:, k, m], gt[c][:, k],
                            start=(k == 0), stop=(k == 1),
                        )
                    if eng == "v":
                        nc.vector.tensor_scalar(
                            out=HS[:, m, sl(c)], in0=pp[:],
                            scalar1=BIA[:, m : m + 1], scalar2=0.0,
                            op0=ALU.add, op1=ALU.max,
                        )
                    else:
                        nc.scalar.activation(
                            out=HS[:, m, sl(c)], in_=pp[:], func=AT.Relu,
                            bias=BIA[:, m : m + 1],
                        )

                def l2(c, m):
                    pq = mmp.tile([128, CHUNK], F32, tag="pp")
                    for k in (0, 1):
                        nc.tensor.matmul(
                            pq[:], W2[:, k, m], HS[:, k, sl(c)],
                            start=(k == 0), stop=(k == 1),
                        )
                    nc.vector.tensor_scalar_add(
                        out=PS[:, m, sl(c)], in0=pq[:],
                        scalar1=BIA[:, 2 + m : 3 + m],
                    )
                    if m == 0:
                        nc.scalar.activation(
                            out=QS[:, m, sl(c)], in_=pq[:], func=AT.Square,
                            bias=BIA[:, 2 + m : 3 + m],
                        )
                    else:
                        nc.vector.tensor_mul(
                            out=QS[:, m, sl(c)], in0=PS[:, m, sl(c)],
                            in1=PS[:, m, sl(c)],
                        )

                def nsq(c):
                    # ||p_s||^2 replicated across all 128 partitions via an
                    # all-ones stationary, then rn = (tau*nsq)^-0.5.
                    nq = nsqp.tile([128, CHUNK], F32, tag="nq")
                    for m in (0, 1):
                        nc.tensor.matmul(
                            nq[:], ones128[:], QS[:, m, sl(c)],
                            start=(m == 0), stop=(m == 1),
                        )
                    lnt = big.tile([128, CHUNK], F32, tag="lnt", bufs=2)
                    nc.scalar.activation(
                        out=lnt[:], in_=nq[:], func=AT.Ln, scale=float(TAU)
                    )
                    nc.scalar.activation(
                        out=RR[:, sl(c)], in_=lnt[:], func=AT.Exp, scale=-0.5
                    )

                def pn(c):
                    nc.vector.tensor_mul(
                        out=PN[:, :, sl(c)], in0=PS[:, :, sl(c)],
                        in1=RR[:, sl(c)].unsqueeze(1).broadcast_to([128, 2, CHUNK]),
                    )

                def sims(g):
                    for q in range(4):
                        n = 4 * g + q
                        if n >= NI:
                            continue
                        a0 = SAMP * n
                        for k in (0, 1):
                            nc.tensor.matmul(
                                sp[32 * q : 32 * q + 32, 64 * g : 64 * g + 64],
                                PN[:, k, a0 : a0 + 32],
                                PN[:, k, a0 + 32 : a0 + 96],
                                start=(k == 0), stop=(k == 1),
                                tile_position=(0, 32 * q),
                            )

                # pad slots (g=3, q>=1) are never written by a matmul;
                # partition-offset accesses may span at most 32 partitions
                for q in (1, 2, 3):
                    nc.vector.memset(sp[32 * q : 32 * q + 32, 192:256], 0.0)

                l1(0, 0, "v"); l1(0, 1, "v")
                l1(1, 0, "s"); l1(1, 1, "s")
                l2(0, 0); l2(0, 1)
                l1(2, 0, "v"); l1(2, 1, "s")
                l2(1, 0); l2(1, 1)
                nsq(0)
                l2(2, 0); l2(2, 1)
                nsq(1); pn(0)
                nsq(2); pn(1); pn(2)
                sims(0); sims(1); sims(2); sims(3)

                # ---- loss tail on [128, 4 groups x 64] ----
                # loss_km = ln(exp(s_ap) + sum_j exp(s_an_j)) - s_ap
                spv = sp[:].rearrange("p (g x) -> p g x", g=4)
                ee = big.tile([128, 256], F32, name="ee")
                nc.scalar.activation(out=ee[:], in_=sp[:], func=AT.Exp)
                eev = ee[:].rearrange("p (g x) -> p g x", g=4)
                ssum = big.tile([128, 4], F32, name="ssum")
                nc.vector.reduce_sum(out=ssum[:], in_=eev[:, :, 32:64], axis=X)
                tt = big.tile([128, 128], F32, name="tt")
                nc.vector.tensor_add(
                    out=tt[:].rearrange("p (g x) -> p g x", g=4),
                    in0=eev[:, :, 0:32],
                    in1=ssum[:].unsqueeze(-1).broadcast_to([128, 4, 32]),
                )
                lg = big.tile([128, 128], F32, name="lg")
                nc.scalar.activation(out=lg[:], in_=tt[:], func=AT.Ln)
                ctb = big.tile([128, 128], F32, name="ctb")
                nc.vector.tensor_sub(
                    out=ctb[:].rearrange("p (g x) -> p g x", g=4),
                    in0=lg[:].rearrange("p (g x) -> p g x", g=4),
                    in1=spv[:, :, 0:32],
                )
                rowr = big.tile([128, 4], F32, name="rowr")
                nc.vector.reduce_sum(
                    out=rowr[:],
                    in_=ctb[:].rearrange("p (g x) -> p g x", g=4),
                    axis=X,
                )
                # sum over the 32 anchor partitions of each block
                lp = lpp.tile([16, 4], F32, tag="lp")
                nc.tensor.matmul(lp[:], MSK[:], rowr[:], start=True, stop=True)
                lout = big.tile([16, 4], F32, name="lout")
                nc.scalar.copy(out=lout[:], in_=lp[:])
                nc.sync.dma_start(out=lossd[:, :], in_=lout[:])

    nc.compile()
    return nc


_NC_CACHE = None


def _get_nc():
    global _NC_CACHE
    if _NC_CACHE is None:
        _NC_CACHE = _build_nc()
    return _NC_CACHE


def _host_prep(feats, w1, b1, w2, b2, anchor_inds, pos_inds, neg_inds):
    """Build the 8 per-core input maps (host-side gather + packing)."""
    ff = np.asarray(feats, np.float32).reshape(N_INST, C, PIX)

    def flat(i):
        i = np.asarray(i)
        return i[..., 0].astype(np.int64) * SIDE + i[..., 1].astype(np.int64)

    idx = np.concatenate(
        [flat(anchor_inds), flat(pos_inds), flat(neg_inds)], axis=1
    )  # [100, 96]
    ntot = N_CORES * NI
    inst = np.arange(ntot) % N_INST  # wrap the 4 pad rows

    G = np.take_along_axis(ff[inst], idx[inst][:, None, :], axis=2)  # [104,256,96]
    G = G.reshape(N_CORES, NI, C, SAMP).transpose(0, 2, 1, 3)  # [8,256,13,96]
    G = G.reshape(N_CORES, 2, 128, NCH, CHUNK).transpose(0, 2, 3, 1, 4)
    gtd = np.ascontiguousarray(G).astype(np_bf16)  # [8,128,3,2,416]

    def wpack(w):
        wa = np.asarray(w, np.float32).reshape(2, 128, 2, 128)  # [m,i,k,p]
        return np.ascontiguousarray(wa.transpose(3, 2, 0, 1)).astype(np_bf16)

    w1d = wpack(w1)
    w2d = wpack(w2)
    b1r = np.asarray(b1, np.float32).reshape(2, 128).T  # [128, 2]
    b2r = np.asarray(b2, np.float32).reshape(2, 128).T
    biasd = np.ascontiguousarray(np.concatenate([b1r, b2r], axis=1))  # [128, 4]
    maskd = np.zeros((128, 16), np.float32)
    for q in range(4):
        maskd[32 * q : 32 * q + 32, q::4] = 1.0

    return [
        {
            "gtd": gtd[c],
            "w1d": w1d,
            "w2d": w2d,
            "biasd": biasd,
            "maskd": maskd,
        }
        for c in range(N_CORES)
    ]


def _finalize(loss_per, gt_mask):
    gt = np.asarray(gt_mask)
    area = gt.reshape(gt.shape[0], -1).sum(axis=1)
    valid = (area > NUM_SAMPLES) & (area < PIX - NUM_SAMPLES)
    n_valid = np.float32(valid.sum())
    if n_valid > 0:
        total = np.float32(np.where(valid, loss_per, 0.0).astype(np.float32).sum())
        out = total / max(n_valid, np.float32(1.0))
    else:
        out = np.float32(0.0)
    return np.float32(out * np.float32(LOSS_WEIGHT))


def kernel(feats, w1, b1, w2, b2, gt_mask, anchor_inds, pos_inds, neg_inds,
           _results_hook=None):
    nc = _get_nc()
    in_maps = _host_prep(feats, w1, b1, w2, b2, anchor_inds, pos_inds, neg_inds)
    res = run_bass_kernel_spmd(nc, in_maps, list(range(N_CORES)))
    if _results_hook is not None:
        _results_hook(res)
    loss_per = np.zeros(N_CORES * NI, np.float32)
    for c in range(N_CORES):
        lo = np.asarray(res.results[c]["loss"], np.float32)  # [16, 4]
        for n in range(NI):
            loss_per[NI * c + n] = lo[n, n // 4]
    loss_per = loss_per[:N_INST] / float(NUM_SAMPLES * NUM_SAMPLES)
    return _finalize(loss_per, gt_mask)
